# revision 1
# baseline (speedup 1.0000x reference)
"""Contrast-depth MSE loss on 8 Trainium2 NeuronCores.

Math: with d = out - label (per image, 32x32 grid flattened to p in [0,1024)),
the loss is an exact quadratic form

    loss = sum_{p,q} C[p,q] * G[p,q] / (B*8*30*30),
    G[p,q] = sum_img d[img,p] * d[img,q]

where C (the contrast-depth-conv quadratic form) is supported on the
diagonals q-p in {0, +-1, +-31, +-32, +-33}.  Each core computes banded
Gram blocks G[128k+r, 128k+c] (c in [0,161)) on the TensorEngine with
PSUM accumulation over its 2048-image shard; the host applies the C
weights to the diagonals and reduces across cores.
"""

import numpy as np

_B = 16384
_H = 32
_W = 32
_P = _H * _W  # 1024 pixels
_NCORES = 8
_BSH = _B // _NCORES  # 2048 images per core
_TILE = 128
_NT = _BSH // _TILE  # 16 tiles per core
_BAND = 161  # 128 + max diagonal offset (33)


def _block_ncols(k: int) -> int:
    return min(_BAND, _P - 128 * k)


_GRAM_COLS = sum(_block_ncols(k) for k in range(8))  # 7*161 + 128 = 1255


def _build_weights() -> np.ndarray:
    """[128, _GRAM_COLS] weights s.t. loss_sum = sum(W * gram_blocks)."""
    C = np.zeros((_P, _P), dtype=np.float64)
    offs = [(a, b) for a in range(3) for b in range(3) if (a, b) != (1, 1)]
    for a, b in offs:
        for i in range(_H - 2):
            for j in range(_W - 2):
                p = (i + a) * _W + (j + b)  # neighbor pixel
                q = (i + 1) * _W + (j + 1)  # center pixel
                C[p, p] += 1.0
                C[q, q] += 1.0
                C[p, q] -= 1.0
                C[q, p] -= 1.0
    W = np.zeros((_TILE, _GRAM_COLS), dtype=np.float64)
    off = 0
    for k in range(8):
        ncols = _block_ncols(k)
        for delta in (0, 1, 31, 32, 33):
            for r in range(_TILE):
                p = 128 * k + r
                q = p + delta
                c = r + delta
                if q >= _P or c >= ncols:
                    continue
                W[r, off + c] = C[p, q] * (1.0 if delta == 0 else 2.0)
        off += ncols
    return W


_WFULL = _build_weights()

_NC_CACHE = None


def _build_nc():
    import concourse.bacc as bacc
    import concourse.mybir as mybir
    import concourse.tile as tile

    nc = bacc.Bacc()
    # Image remap: host shard [2048, 1024] is viewed as [128, 16*1024] where
    # partition p holds images 16p..16p+15 (a free reshape of the same
    # buffer).  Each partition's DMA reads are then long contiguous runs.
    _FREE = _NT * _P  # 16384 f32 per partition
    out_d = nc.dram_tensor("out", [_TILE, _FREE], mybir.dt.float32, kind="ExternalInput")
    lab_d = nc.dram_tensor("label", [_TILE, _FREE], mybir.dt.float32, kind="ExternalInput")
    gram_d = nc.dram_tensor(
        "gram", [_TILE, _GRAM_COLS], mybir.dt.float32, kind="ExternalOutput"
    )

    # ramped chunk sizes (in image-tiles): small chunks at both ends (early
    # PE start, short tail), big chunks (16KB runs/partition) mid-stream.
    _CHUNKS = [4, 4, 4, 2, 1, 1]
    assert sum(_CHUNKS) == _NT

    with tile.TileContext(nc) as tc:
        with (
            tc.tile_pool(name="buf", bufs=1) as buf_pool,
            tc.tile_pool(name="ps", bufs=1, space="PSUM") as psum_pool,
        ):
            grams = []
            offs = []
            off = 0
            for k in range(8):
                ncols = _block_ncols(k)
                grams.append(
                    psum_pool.tile(
                        [_TILE, ncols], mybir.dt.float32, tag=f"g{k}", name=f"g{k}"
                    )
                )
                offs.append(off)
                off += ncols

            # persistent SBUF buffers: every chunk DMA can enqueue
            # immediately; no pool-slot rotation ever blocks the DMA stream.
            o = buf_pool.tile([_TILE, _FREE], mybir.dt.float32, tag="o", name="o")
            lb = buf_pool.tile([_TILE, _FREE], mybir.dt.float32, tag="l", name="l")
            d = buf_pool.tile([_TILE, _FREE], mybir.dt.bfloat16, tag="d", name="d")

            tbase = 0
            for tpc in _CHUNKS:
                cf = tpc * _P
                c0 = tbase * _P
                nc.sync.dma_start(out=o[:, c0 : c0 + cf], in_=out_d[:, c0 : c0 + cf])
                nc.scalar.dma_start(
                    out=lb[:, c0 : c0 + cf], in_=lab_d[:, c0 : c0 + cf]
                )
                for tt in range(tpc):
                    base = c0 + tt * _P
                    nc.vector.tensor_sub(
                        out=d[:, base : base + _P],
                        in0=o[:, base : base + _P],
                        in1=lb[:, base : base + _P],
                    )
                    first = tbase + tt == 0
                    last = tbase + tt == _NT - 1
                    for k in range(8):
                        ncols = _block_ncols(k)
                        nc.tensor.matmul(
                            grams[k][:, :ncols],
                            lhsT=d[:, base + 128 * k : base + 128 * k + 128],
                            rhs=d[:, base + 128 * k : base + 128 * k + ncols],
                            start=first,
                            stop=last,
                        )
                tbase += tpc

            result = buf_pool.tile(
                [_TILE, _GRAM_COLS], mybir.dt.float32, tag="r", name="r"
            )
            for k in range(8):
                ncols = _block_ncols(k)
                dst = result[:, offs[k] : offs[k] + ncols]
                nc.vector.tensor_copy(out=dst, in_=grams[k][:])
            nc.sync.dma_start(out=gram_d[:], in_=result[:])
    nc.finalize()
    return nc


def _run(out, label, trace=False):
    from concourse.bass_utils import run_bass_kernel_spmd

    global _NC_CACHE
    out = np.ascontiguousarray(np.asarray(out), dtype=np.float32).reshape(_B, _P)
    label = np.ascontiguousarray(np.asarray(label), dtype=np.float32).reshape(_B, _P)
    if _NC_CACHE is None:
        _NC_CACHE = _build_nc()
    in_maps = [
        {
            "out": out[i * _BSH : (i + 1) * _BSH],
            "label": label[i * _BSH : (i + 1) * _BSH],
        }
        for i in range(_NCORES)
    ]
    res = run_bass_kernel_spmd(
        _NC_CACHE, in_maps, core_ids=list(range(_NCORES)), trace=trace
    )
    total = 0.0
    for r in res.results:
        total += float((_WFULL * r["gram"].astype(np.float64)).sum())
    loss = total / (_B * 8 * (_H - 2) * (_W - 2))
    return np.asarray(np.float32(loss)), res


def kernel(out, label):
    loss, _ = _run(out, label, trace=False)
    return loss

